# revision 9
# baseline (speedup 1.0000x reference)
"""DifferentialAttentionBlock on 8 NeuronCores.

Sharding: DP on batch (cores 0-3 = batch 0, 4-7 = batch 1) x TP on heads
(4 heads per core). Per-core dataflow (transposed-activation layout):
  qT/kT/vT (host-transposed) -> projections q1T/q2T/k1T/k2T [128c, S] and
  vv [S, 256] (+ones col) -> per-head transposed scores (32x128 row-tiled
  PE) -> exp (ACT, scale=1/8, max-free) -> A@V natural form with fused
  row-sums -> per-partition softmax normalize + lambda combine -> attn
  [S, 256] -> PE transpose -> bf16 AllToAll (heads x sq-block exchange)
  -> full-Wo bf16 matmul on each core's 256 output rows.
Output bias bo is added on host after the gather.
"""

import math
import numpy as np

B, S, D = 2, 1024, 1024
H = 16
DH = 32          # q/k half head dim
DK = 64          # v head dim
HPC = 4          # heads per core
CPB = 4          # cores per batch (TP group size)
NCORES = 8
LAMBDA_INIT = 0.8 - 0.6 * math.exp(-0.3 * (1 - 1))
NSK = S // 128   # 8 s_k tiles
CHW = 512        # sq chunk width
NCH = S // CHW   # 2 chunks
RG = [[0, 1, 2, 3], [4, 5, 6, 7]]

PROFILE = False
LAST_EXEC_NS = None
LAST_RESULTS = None

_cache = {}


def _try_install_ntff_hook():
    try:
        import sys, types
        import antenv
        try:
            import antenv.axon_hooks  # noqa: F401
            return
        except ImportError:
            pass
        mod = types.ModuleType("antenv.axon_hooks")
        mod._hook = None
        mod.set_axon_ntff_profile_hook = lambda h: setattr(mod, "_hook", h)
        mod.get_axon_ntff_profile_hook = lambda: mod._hook
        sys.modules["antenv.axon_hooks"] = mod
        antenv.axon_hooks = mod
        from trn_agent_boot.trn_boot import _ntff_profile_via_ctypes
        mod._hook = _ntff_profile_via_ctypes('/opt/axon/libaxon_pjrt.so')
    except Exception:
        pass


def _build(causal: bool):
    import concourse.bacc as bacc
    import concourse.mybir as mybir
    import concourse.tile as tile

    dt = mybir.dt
    f32, f32r, bf16 = dt.float32, dt.float32r, dt.bfloat16
    AF = mybir.ActivationFunctionType
    OP = mybir.AluOpType

    nc = bacc.Bacc("TRN2", target_bir_lowering=False, debug=False,
                   num_devices=NCORES)

    def inp(name, shape, d=f32):
        return nc.dram_tensor(name, shape, d, kind="ExternalInput")

    qT = inp("qT", [D, S], f32r)
    kT = inp("kT", [D, S], f32r)
    vT = inp("vT", [D, S], f32r)
    Wq1 = inp("Wq1", [D, 128], f32r);  Wq2 = inp("Wq2", [D, 128], f32r)
    Wk1 = inp("Wk1", [D, 128], f32r);  Wk2 = inp("Wk2", [D, 128], f32r)
    Wv = inp("Wv", [D, 256], f32r)
    Wob = inp("Wob", [D, 256], bf16)
    bq1 = inp("bq1", [128, 1]);  bq2 = inp("bq2", [128, 1])
    bk1 = inp("bk1", [128, 1]);  bk2 = inp("bk2", [128, 1])
    bv = inp("bv", [1, 256], f32r)
    ones_in = inp("ones1", [1, 128], f32r)
    ident = inp("ident", [128, 128])
    triu = inp("triu", [128, 128], bf16)
    lamv = inp("lamv", [128, 1])
    maskT = None if causal else inp("maskT", [S, S])
    out_ext = nc.dram_tensor("out", [S, 256], f32, kind="ExternalOutput")

    r32 = lambda ap: ap.bitcast(f32r)

    with tile.TileContext(nc) as tc:
        with (
            tc.tile_pool(name="const", bufs=1) as cpool,
            tc.tile_pool(name="wts", bufs=1) as wpool,
            tc.tile_pool(name="proj", bufs=1) as ppool,
            tc.tile_pool(name="acts", bufs=2) as apool,
            tc.tile_pool(name="edata", bufs=1) as epool,
            tc.tile_pool(name="attn", bufs=1) as atpool,
            tc.tile_pool(name="small", bufs=4) as spool,
            tc.tile_pool(name="outs", bufs=2) as opool,
            tc.tile_pool(name="dram", bufs=1, space="DRAM") as dpool,
        ):
            # ---- constants / weights ----
            ident_sb = cpool.tile([128, 128], f32, tag="ident")
            nc.sync.dma_start(ident_sb[:], ident[:, :])
            triu_sb = cpool.tile([128, 128], bf16, tag="triu")
            nc.sync.dma_start(triu_sb[:], triu[:, :])
            lam_sb = cpool.tile([128, 1], f32, tag="lamv")
            nc.sync.dma_start(lam_sb[:], lamv[:, :])
            ones1 = cpool.tile([1, 128], f32r, tag="ones1")
            nc.sync.dma_start(ones1[:], ones_in[:, :])
            bsb = {}
            for name, t in (("bq1", bq1), ("bq2", bq2), ("bk1", bk1),
                            ("bk2", bk2)):
                bsb[name] = cpool.tile([128, 1], f32, tag=name, name=name)
                nc.sync.dma_start(bsb[name][:], t[:, :])
            bv_sb = cpool.tile([1, 256], f32r, tag="bv")
            nc.sync.dma_start(bv_sb[:], bv[:, :])

            wsb = {}
            for name, t, w in (("Wq1", Wq1, 128), ("Wq2", Wq2, 128),
                               ("Wk1", Wk1, 128), ("Wk2", Wk2, 128),
                               ("Wv", Wv, 256)):
                wsb[name] = wpool.tile([128, 8 * w], f32r, tag=name, name=name)
                for d in range(8):
                    nc.sync.dma_start(wsb[name][:, d * w:(d + 1) * w],
                                      t[d * 128:(d + 1) * 128, :])
            wo_sb = []
            for k in range(8):
                wt = wpool.tile([128, 256], bf16, tag=f"Wob{k}", name=f"Wob{k}")
                nc.sync.dma_start(wt[:], Wob[k * 128:(k + 1) * 128, :])
                wo_sb.append(wt)

            # ---- phase B: q/k projections (transposed layout) ----
            with tc.tile_pool(name="psB", bufs=1, space="PSUM") as psB:
                pq1 = psB.tile([128, S], f32, tag="q1")
                pq2 = psB.tile([128, S], f32, tag="q2")
                pk1 = psB.tile([128, S], f32, tag="k1")
                pk2 = psB.tile([128, S], f32, tag="k2")
                for d in range(8):
                    qTd = apool.tile([128, S], f32r, tag="qTd")
                    nc.sync.dma_start(qTd[:], qT[d * 128:(d + 1) * 128, :])
                    kTd = apool.tile([128, S], f32r, tag="kTd")
                    nc.sync.dma_start(kTd[:], kT[d * 128:(d + 1) * 128, :])
                    for ps, wname, src in ((pq1, "Wq1", qTd), (pq2, "Wq2", qTd),
                                           (pk1, "Wk1", kTd), (pk2, "Wk2", kTd)):
                        lhsT = r32(wsb[wname][:, d * 128:(d + 1) * 128])
                        for half in range(2):
                            nc.tensor.matmul(
                                ps[:, half * 512:(half + 1) * 512], lhsT,
                                r32(src[:, half * 512:(half + 1) * 512]),
                                start=(d == 0), stop=(d == 7))
                q1T = ppool.tile([128, S], f32r, tag="q1T")
                q2T = ppool.tile([128, S], f32r, tag="q2T")
                k1T = ppool.tile([128, S], f32r, tag="k1T")
                k2T = ppool.tile([128, S], f32r, tag="k2T")
                nc.scalar.activation(q1T[:], pq1[:], AF.Identity, bias=bsb["bq1"][:])
                nc.vector.tensor_scalar(q2T[:], pq2[:], bsb["bq2"][:],
                                        None, OP.add)
                nc.scalar.activation(k1T[:], pk1[:], AF.Identity, bias=bsb["bk1"][:])
                nc.vector.tensor_scalar(k2T[:], pk2[:], bsb["bk2"][:],
                                        None, OP.add)

            # ---- phase C: vv projection (natural layout) + ones cols ----
            # vvo layout: [128, 8*260] bf16; block i: 4 heads x (64 vv + 1 one)
            vvo = ppool.tile([128, 8 * 260], bf16, tag="vvo")
            with tc.tile_pool(name="psC", bufs=1, space="PSUM") as psC:
                pvv = [psC.tile([128, 256], f32, tag=f"vv{i}", name=f"vv{i}")
                       for i in range(8)]
                for d in range(8):
                    vTd = apool.tile([128, S], f32r, tag="vTd")
                    nc.sync.dma_start(vTd[:], vT[d * 128:(d + 1) * 128, :])
                    for i in range(8):
                        nc.tensor.matmul(
                            pvv[i][:], r32(vTd[:, i * 128:(i + 1) * 128]),
                            r32(wsb["Wv"][:, d * 256:(d + 1) * 256]),
                            start=(d == 0), stop=False)
                for i in range(8):
                    nc.tensor.matmul(pvv[i][:], r32(ones1[:]), r32(bv_sb[:]),
                                     start=False, stop=True)
                    blk = vvo[:, i * 260:(i + 1) * 260]
                    blk3 = blk.rearrange("p (h c) -> p h c", c=65)
                    nc.vector.tensor_copy(
                        blk3[:, :, 0:64],
                        pvv[i].rearrange("p (h c) -> p h c", c=64))
                    nc.vector.memset(blk3[:, :, 64:65], 1.0)

            # ---- phase D: attention ----
            attn_t = [atpool.tile([128, 256], f32, tag=f"attn{j}", name=f"attn{j}")
                      for j in range(8)]
            kproj = (k1T, k2T)
            qproj = (q1T, q2T)
            with (
                tc.tile_pool(name="psS", bufs=1, space="PSUM") as psS,
                tc.tile_pool(name="psAV", bufs=1, space="PSUM") as psAV,
                tc.tile_pool(name="mloc", bufs=2) as mpool,
            ):
                for c in range(NCH):
                    cs = c * CHW
                    etiles = {}
                    ilist = list(range(min(NSK, (cs + CHW) // 128))) if causal \
                        else list(range(NSK))
                    for i in ilist:
                        lo = max(cs, 128 * i) if causal else cs
                        n = cs + CHW - lo
                        if not causal:
                            mT_sb = mpool.tile([128, CHW], f32, tag="maskT")
                            nc.sync.dma_start(
                                mT_sb[:],
                                maskT[i * 128:(i + 1) * 128, cs:cs + CHW])
                        for m in range(2):
                            for h in range(4):
                                ps = psS.tile([128, CHW], f32, tag=f"s{h}")
                                nc.tensor.matmul(
                                    ps[:, 0:n],
                                    r32(kproj[m][32 * h:32 * h + 32,
                                                 i * 128:(i + 1) * 128]),
                                    r32(qproj[m][32 * h:32 * h + 32,
                                                 lo:cs + CHW]),
                                    start=True, stop=True,
                                    tile_position=(32 * h, 0))
                                if not causal:
                                    nc.vector.tensor_tensor(
                                        ps[:, 0:n], ps[:, 0:n], mT_sb[:, 0:n],
                                        OP.add)
                                e = epool.tile([128, CHW], bf16,
                                               tag=f"e{m}h{h}i{i}",
                                               name=f"e{m}h{h}i{i}c{c}")
                                etiles[(m, h, i)] = e
                                nc.scalar.activation(
                                    e[:, lo - cs:CHW], ps[:, 0:n], AF.Exp,
                                    scale=0.125)
                                if causal and 128 * i >= cs:
                                    off = 128 * i - cs
                                    nc.vector.tensor_tensor(
                                        e[:, off:off + 128],
                                        e[:, off:off + 128], triu_sb[:],
                                        OP.mult)
                    for j in range(c * 4, c * 4 + 4):
                        jo = 128 * j - cs
                        o1 = psAV.tile([128, 260], f32, tag="o1")
                        o2 = psAV.tile([128, 260], f32, tag="o2")
                        ij = [i for i in ilist if (i <= j if causal else True)]
                        for h in range(4):
                            for om, o in ((0, o1), (1, o2)):
                                for x, i in enumerate(ij):
                                    nc.tensor.matmul(
                                        o[:, 65 * h:65 * h + 65],
                                        etiles[(om, h, i)][:, jo:jo + 128],
                                        vvo[:, 260 * i + 65 * h:
                                            260 * i + 65 * h + 65],
                                        start=(x == 0), stop=(x == len(ij) - 1))
                        for h in range(4):
                            r1 = spool.tile([128, 1], f32, tag="r1")
                            nc.vector.reciprocal(r1[:],
                                                 o1[:, 65 * h + 64:65 * h + 65])
                            r2 = spool.tile([128, 1], f32, tag="r2")
                            nc.vector.reciprocal(r2[:],
                                                 o2[:, 65 * h + 64:65 * h + 65])
                            a_sl = attn_t[j][:, 64 * h:64 * h + 64]
                            nc.vector.tensor_scalar(
                                a_sl, o1[:, 65 * h:65 * h + 64], r1[:], None,
                                OP.mult)
                            t2 = spool.tile([128, 64], f32, tag="t2")
                            nc.vector.tensor_scalar(
                                t2[:], o2[:, 65 * h:65 * h + 64], r2[:],
                                lam_sb[:], OP.mult, OP.mult)
                            nc.vector.tensor_tensor(a_sl, a_sl, t2[:],
                                                    OP.subtract)

            # ---- phase E: transpose attn -> attnT (bf16) -> A2A ----
            aT_sb = [ppool.tile([128, S], bf16, tag=f"aT{kk}", name=f"aT{kk}")
                     for kk in range(2)]
            with tc.tile_pool(name="psT", bufs=2, space="PSUM") as psT:
                for j in range(8):
                    for kk in range(2):
                        tp = psT.tile([128, 128], f32, tag="tp")
                        nc.tensor.transpose(
                            tp[:], attn_t[j][:, 128 * kk:128 * (kk + 1)],
                            ident_sb[:])
                        dst = aT_sb[kk][:, 128 * j:128 * (j + 1)]
                        if (j + kk) % 2 == 0:
                            nc.scalar.copy(dst, tp[:])
                        else:
                            nc.vector.tensor_copy(dst, tp[:])
            bounce = dpool.tile([256, S], bf16)
            aT_full = dpool.tile([S, S], bf16)
            for kk in range(2):
                nc.sync.dma_start(bounce[128 * kk:128 * (kk + 1), :],
                                  aT_sb[kk][:])
            nc.gpsimd.collective_compute(
                "AllGather", mybir.AluOpType.bypass, replica_groups=RG,
                ins=[bounce.opt()], outs=[aT_full.opt()])

            # ---- phase F: col-sharded Wo on full sq (out[:, 256g:+256]) ----
            myT = []
            for k in range(8):
                t = ppool.tile([128, S], bf16, tag=f"myT{k}", name=f"myT{k}")
                nc.sync.dma_start(t[:], aT_full[128 * k:128 * (k + 1), :])
                myT.append(t)
            with tc.tile_pool(name="psW", bufs=3, space="PSUM") as psW:
                for mt in range(8):
                    wps = psW.tile([128, 256], f32, tag="wo")
                    for k in range(8):
                        nc.tensor.matmul(
                            wps[:], myT[k][:, 128 * mt:128 * (mt + 1)],
                            wo_sb[k][:],
                            start=(k == 0), stop=(k == 7))
                    osb = opool.tile([128, 256], f32, tag="osb")
                    if mt % 2 == 0:
                        nc.scalar.copy(osb[:], wps[:])
                    else:
                        nc.vector.tensor_copy(osb[:], wps[:])
                    nc.sync.dma_start(out_ext[128 * mt:128 * (mt + 1), :],
                                      osb[:])

    nc.compile()
    return nc


def kernel(**inputs):
    global LAST_EXEC_NS
    import ml_dtypes

    q = np.asarray(inputs["q"], dtype=np.float32)
    k = np.asarray(inputs["k"], dtype=np.float32)
    v = np.asarray(inputs["v"], dtype=np.float32)
    mask = np.asarray(inputs["mask"])
    f32 = np.float32
    Wq1f = np.asarray(inputs["Wq1"], f32); Wq2f = np.asarray(inputs["Wq2"], f32)
    Wk1f = np.asarray(inputs["Wk1"], f32); Wk2f = np.asarray(inputs["Wk2"], f32)
    Wvf = np.asarray(inputs["Wv"], f32);   Wof = np.asarray(inputs["Wo"], f32)
    bq1f = np.asarray(inputs["bq1"], f32); bq2f = np.asarray(inputs["bq2"], f32)
    bk1f = np.asarray(inputs["bk1"], f32); bk2f = np.asarray(inputs["bk2"], f32)
    bvf = np.asarray(inputs["bv"], f32);   bof = np.asarray(inputs["bo"], f32)
    lam = float(np.exp(float(inputs["lq1"][0]) * float(inputs["lk1"][0]))
                - np.exp(float(inputs["lq2"][0]) * float(inputs["lk2"][0]))
                + LAMBDA_INIT)

    mk = (mask.reshape(B, S, S) != 0)
    causal = bool((mk == np.tril(np.ones((S, S), bool))[None]).all())

    key = "causal" if causal else "general"
    if key not in _cache:
        _cache[key] = _build(causal)
    nc = _cache[key]

    qT = [np.ascontiguousarray(q[b].T) for b in range(B)]
    kTl = [np.ascontiguousarray(k[b].T) for b in range(B)]
    vTl = [np.ascontiguousarray(v[b].T) for b in range(B)]
    Wob = Wof.astype(ml_dtypes.bfloat16)
    ident = np.eye(128, dtype=f32)
    triu = np.triu(np.ones((128, 128))).astype(ml_dtypes.bfloat16)
    lamv = np.full((128, 1), lam, f32)
    maskTs = None
    if not causal:
        maskTs = [np.ascontiguousarray(
            np.where(mk[b], np.float32(0), np.float32(-1e9)).T)
            for b in range(B)]

    in_maps = []
    for c in range(NCORES):
        b, g = divmod(c, CPB)
        im = dict(
            qT=qT[b], kT=kTl[b], vT=vTl[b],
            Wq1=np.ascontiguousarray(Wq1f[:, 128 * g:128 * (g + 1)]),
            Wq2=np.ascontiguousarray(Wq2f[:, 128 * g:128 * (g + 1)]),
            Wk1=np.ascontiguousarray(Wk1f[:, 128 * g:128 * (g + 1)]),
            Wk2=np.ascontiguousarray(Wk2f[:, 128 * g:128 * (g + 1)]),
            Wv=np.ascontiguousarray(Wvf[:, 256 * g:256 * (g + 1)]),
            Wob=np.ascontiguousarray(Wob[:, 256 * g:256 * (g + 1)]),
            bq1=np.ascontiguousarray(bq1f[128 * g:128 * (g + 1)]).reshape(128, 1),
            bq2=np.ascontiguousarray(bq2f[128 * g:128 * (g + 1)]).reshape(128, 1),
            bk1=np.ascontiguousarray(bk1f[128 * g:128 * (g + 1)]).reshape(128, 1),
            bk2=np.ascontiguousarray(bk2f[128 * g:128 * (g + 1)]).reshape(128, 1),
            bv=np.ascontiguousarray(bvf[256 * g:256 * (g + 1)]).reshape(1, 256),
            ident=ident, triu=triu, lamv=lamv,
            ones1=np.ones((1, 128), f32),
        )
        if not causal:
            im["maskT"] = maskTs[b]
        in_maps.append(im)

    from concourse.bass_utils import run_bass_kernel_spmd
    if PROFILE:
        _try_install_ntff_hook()
        res = run_bass_kernel_spmd(nc, in_maps, list(range(NCORES)),
                                   trace=True)
        LAST_EXEC_NS = res.exec_time_ns
        globals()["LAST_RESULTS"] = res
    else:
        res = run_bass_kernel_spmd(nc, in_maps, list(range(NCORES)))

    out = np.empty((B, S, D), np.float32)
    for c in range(NCORES):
        b, g = divmod(c, CPB)
        out[b, :, 256 * g:256 * (g + 1)] = res.results[c]["out"]
    out += bof[None, None, :]
    return out


# revision 12
# speedup vs baseline: 1.0141x; 1.0141x over previous
"""DifferentialAttentionBlock on 8 NeuronCores.

Sharding: DP on batch (cores 0-3 = batch 0, 4-7 = batch 1) x TP on heads
(4 heads per core) for everything through attention; then an 8-rank
AllGather of bf16 attnT shards and a 128-column Wo shard per core over
both batches (keeps the program free of per-core offsets).

Per-core dataflow (transposed-activation layout):
  qT/kT/vT (host-transposed) -> projections q1T/q2T/k1T/k2T [128c, S]
  (fp32r) and vv [S, 256] (+ones col, bf16) -> per-head transposed
  scores (32x128 row-tiled PE, fp32r) -> exp (ACT, scale=1/8, max-free)
  -> bf16 A@V in outT form (vv stationary; fused colsum row) ->
  reciprocal + gpsimd partition-broadcast normalize + lambda combine
  straight into attnT bf16 -> chunked 8-rank AllGather -> Wo col-shard
  bf16 matmul -> out [2*S, 128] per core.  Output bias bo on host.
"""

import math
import numpy as np

B, S, D = 2, 1024, 1024
H = 16
DH = 32          # q/k half head dim
DK = 64          # v head dim
HPC = 4          # heads per core
CPB = 4          # cores per batch (TP group size)
NCORES = 8
LAMBDA_INIT = 0.8 - 0.6 * math.exp(-0.3 * (1 - 1))
NSK = S // 128   # 8 s_k tiles
CHW = 512        # sq chunk width
NCH = S // CHW   # 2 chunks
RG8 = [list(range(8))]

PROFILE = False
LAST_EXEC_NS = None
LAST_RESULTS = None

_cache = {}


def _try_install_ntff_hook():
    try:
        import sys, types
        import antenv
        try:
            import antenv.axon_hooks  # noqa: F401
            return
        except ImportError:
            pass
        mod = types.ModuleType("antenv.axon_hooks")
        mod._hook = None
        mod.set_axon_ntff_profile_hook = lambda h: setattr(mod, "_hook", h)
        mod.get_axon_ntff_profile_hook = lambda: mod._hook
        sys.modules["antenv.axon_hooks"] = mod
        antenv.axon_hooks = mod
        from trn_agent_boot.trn_boot import _ntff_profile_via_ctypes
        mod._hook = _ntff_profile_via_ctypes('/opt/axon/libaxon_pjrt.so')
    except Exception:
        pass


def _build(causal: bool):
    import concourse.bacc as bacc
    import concourse.mybir as mybir
    import concourse.tile as tile

    dt = mybir.dt
    f32, f32r, bf16 = dt.float32, dt.float32r, dt.bfloat16
    AF = mybir.ActivationFunctionType
    OP = mybir.AluOpType

    nc = bacc.Bacc("TRN2", target_bir_lowering=False, debug=False,
                   num_devices=NCORES)

    def inp(name, shape, d=f32):
        return nc.dram_tensor(name, shape, d, kind="ExternalInput")

    qT = inp("qT", [D, S], f32r)
    kT = inp("kT", [D, S], f32r)
    vT = inp("vT", [D, S], f32r)
    Wq1 = inp("Wq1", [D, 128], f32r);  Wq2 = inp("Wq2", [D, 128], f32r)
    Wk1 = inp("Wk1", [D, 128], f32r);  Wk2 = inp("Wk2", [D, 128], f32r)
    Wv = inp("Wv", [D, 256], f32r)
    Wob = inp("Wob", [D, 128], bf16)          # my 128 output columns
    bq1 = inp("bq1", [128, 1]);  bq2 = inp("bq2", [128, 1])
    bk1 = inp("bk1", [128, 1]);  bk2 = inp("bk2", [128, 1])
    bv = inp("bv", [1, 256], f32r)
    ones_in = inp("ones1", [1, 128], f32r)
    triu = inp("triu", [128, 128], bf16)
    lamv = inp("lamv", [128, 1])
    maskT = None if causal else inp("maskT", [S, S])
    out_ext = nc.dram_tensor("out", [B * S, 128], f32, kind="ExternalOutput")

    with tile.TileContext(nc) as tc:
        with (
            tc.tile_pool(name="const", bufs=1) as cpool,
            tc.tile_pool(name="wts", bufs=1) as wpool,
            tc.tile_pool(name="proj", bufs=1) as ppool,
            tc.tile_pool(name="acts", bufs=2) as apool,
            tc.tile_pool(name="edata", bufs=1) as epool,
            tc.tile_pool(name="small", bufs=2) as spool,
            tc.tile_pool(name="outs", bufs=2) as opool,
            tc.tile_pool(name="dram", bufs=1, space="DRAM") as dpool,
        ):
            # ---- constants + q/k projection weights first ----
            triu_sb = cpool.tile([128, 128], bf16, tag="triu")
            nc.sync.dma_start(triu_sb[:], triu[:, :])
            lam_sb = cpool.tile([128, 1], f32, tag="lamv")
            nc.sync.dma_start(lam_sb[:], lamv[:, :])
            ones1 = cpool.tile([1, 128], f32r, tag="ones1")
            nc.sync.dma_start(ones1[:], ones_in[:, :])
            bsb = {}
            for name, t in (("bq1", bq1), ("bq2", bq2), ("bk1", bk1),
                            ("bk2", bk2)):
                bsb[name] = cpool.tile([128, 1], f32, tag=name, name=name)
                nc.sync.dma_start(bsb[name][:], t[:, :])
            bv_sb = cpool.tile([1, 256], f32r, tag="bv")
            nc.sync.dma_start(bv_sb[:], bv[:, :])

            wsb = {}
            for name, t in (("Wq1", Wq1), ("Wq2", Wq2),
                            ("Wk1", Wk1), ("Wk2", Wk2)):
                wsb[name] = wpool.tile([128, 8 * 128], f32r, tag=name,
                                       name=name)
                # one 3-D DMA: dram [8, 128, 128] -> sbuf [128, (8, 128)]
                nc.sync.dma_start(
                    wsb[name][:].rearrange("p (d c) -> p d c", d=8),
                    t.rearrange("(d p) c -> p d c", p=128))

            # ---- phase B: q/k projections (transposed layout) ----
            with tc.tile_pool(name="psB", bufs=1, space="PSUM") as psB:
                pq1 = psB.tile([128, S], f32, tag="q1")
                pq2 = psB.tile([128, S], f32, tag="q2")
                pk1 = psB.tile([128, S], f32, tag="k1")
                pk2 = psB.tile([128, S], f32, tag="k2")
                for d in range(8):
                    qTd = apool.tile([128, S], f32r, tag="qTd")
                    nc.sync.dma_start(qTd[:], qT[d * 128:(d + 1) * 128, :])
                    kTd = apool.tile([128, S], f32r, tag="kTd")
                    nc.sync.dma_start(kTd[:], kT[d * 128:(d + 1) * 128, :])
                    for ps, wname, src in ((pq1, "Wq1", qTd), (pq2, "Wq2", qTd),
                                           (pk1, "Wk1", kTd), (pk2, "Wk2", kTd)):
                        lhsT = wsb[wname][:, d * 128:(d + 1) * 128]
                        for half in range(2):
                            nc.tensor.matmul(
                                ps[:, half * 512:(half + 1) * 512], lhsT,
                                src[:, half * 512:(half + 1) * 512],
                                start=(d == 0), stop=(d == 7))
                q1T = ppool.tile([128, S], f32r, tag="q1T")
                q2T = ppool.tile([128, S], f32r, tag="q2T")
                k1T = ppool.tile([128, S], f32r, tag="k1T")
                k2T = ppool.tile([128, S], f32r, tag="k2T")
                nc.scalar.activation(q1T[:], pq1[:], AF.Identity,
                                     bias=bsb["bq1"][:])
                nc.vector.tensor_scalar(q2T[:], pq2[:], bsb["bq2"][:],
                                        None, OP.add)
                nc.scalar.activation(k1T[:], pk1[:], AF.Identity,
                                     bias=bsb["bk1"][:])
                nc.vector.tensor_scalar(k2T[:], pk2[:], bsb["bk2"][:],
                                        None, OP.add)

            # ---- phase C: vv projection (natural layout) + ones cols ----
            wv_sb = wpool.tile([128, 8 * 256], f32r, tag="Wv")
            nc.sync.dma_start(wv_sb[:].rearrange("p (d c) -> p d c", d=8),
                              Wv.rearrange("(d p) c -> p d c", p=128))
            # vvo layout: [128, 8*260] bf16; block i: 4 heads x (64 vv + 1 one)
            vvo = ppool.tile([128, 8 * 260], bf16, tag="vvo")
            with tc.tile_pool(name="psC", bufs=1, space="PSUM") as psC:
                pvv = [psC.tile([128, 256], f32, tag=f"vv{i}", name=f"vv{i}")
                       for i in range(8)]
                for d in range(8):
                    vTd = apool.tile([128, S], f32r, tag="vTd")
                    nc.sync.dma_start(vTd[:], vT[d * 128:(d + 1) * 128, :])
                    for i in range(8):
                        nc.tensor.matmul(
                            pvv[i][:], vTd[:, i * 128:(i + 1) * 128],
                            wv_sb[:, d * 256:(d + 1) * 256],
                            start=(d == 0), stop=False)
                for i in range(8):
                    nc.tensor.matmul(pvv[i][:], ones1[:], bv_sb[:],
                                     start=False, stop=True)
                    blk = vvo[:, i * 260:(i + 1) * 260]
                    blk3 = blk.rearrange("p (h c) -> p h c", c=65)
                    nc.vector.tensor_copy(
                        blk3[:, :, 0:64],
                        pvv[i].rearrange("p (h c) -> p h c", c=64))
                    nc.vector.memset(blk3[:, :, 64:65], 1.0)

            # Wo col-shard (needed only in phase F; load early, off crit path)
            wo_sb = wpool.tile([128, 8 * 128], bf16, tag="Wob")
            nc.sync.dma_start(wo_sb[:].rearrange("p (d c) -> p d c", d=8),
                              Wob.rearrange("(d p) c -> p d c", p=128))

            # ---- phase D: attention; attnT assembled directly ----
            aT_sb = [ppool.tile([128, S], bf16, tag=f"aT{kk}", name=f"aT{kk}")
                     for kk in range(2)]
            kproj = (k1T, k2T)
            qproj = (q1T, q2T)
            bounce = [dpool.tile([256, CHW], bf16, name=f"bounce{c}")
                      for c in range(NCH)]
            ag_out = [dpool.tile([NCORES * 256, CHW], bf16, name=f"ag{c}")
                      for c in range(NCH)]

            def wo_chunk(c):
                """Wo matmuls for sq cols [c*CHW, (c+1)*CHW) of the gathered
                attnT; emits out rows for those sq positions, both batches."""
                myt = opool.tile([128, 16 * CHW], bf16, tag="myt",
                                 name=f"myt{c}", bufs=1)
                # dram [16, 128, CHW] -> sbuf [128, (16, CHW)]
                nc.sync.dma_start(
                    myt[:].rearrange("p (k x) -> p k x", k=16),
                    ag_out[c].rearrange("(k p) x -> p k x", p=128))
                with tc.tile_pool(name=f"psW{c}", bufs=2,
                                  space="PSUM") as psW:
                    for b in range(B):
                        for mt in range(CHW // 128):
                            wps = psW.tile([128, 128], f32, tag="wo",
                                           name=f"wo{c}{b}{mt}")
                            for k in range(8):
                                kk = b * 8 + k
                                nc.tensor.matmul(
                                    wps[:],
                                    myt[:, CHW * kk + 128 * mt:
                                        CHW * kk + 128 * (mt + 1)],
                                    wo_sb[:, 128 * k:128 * (k + 1)],
                                    start=(k == 0), stop=(k == 7))
                            osb = opool.tile([128, 128], f32, tag="osb",
                                             name=f"osb{c}{b}{mt}")
                            if mt % 2 == 0:
                                nc.scalar.copy(osb[:], wps[:])
                            else:
                                nc.vector.tensor_copy(osb[:], wps[:])
                            row = b * S + c * CHW + 128 * mt
                            nc.sync.dma_start(out_ext[row:row + 128, :],
                                              osb[:])

            with (
                tc.tile_pool(name="psS", bufs=1, space="PSUM") as psS,
                tc.tile_pool(name="psAV", bufs=1, space="PSUM") as psAV,
                tc.tile_pool(name="mloc", bufs=2) as mpool,
            ):
                for c in range(NCH):
                    cs = c * CHW
                    etiles = {}
                    ilist = list(range(min(NSK, (cs + CHW) // 128))) if causal \
                        else list(range(NSK))
                    for i in ilist:
                        lo = max(cs, 128 * i) if causal else cs
                        n = cs + CHW - lo
                        if not causal:
                            mT_sb = mpool.tile([128, CHW], f32, tag="maskT",
                                               name=f"mT{c}{i}")
                            nc.sync.dma_start(
                                mT_sb[:],
                                maskT[i * 128:(i + 1) * 128, cs:cs + CHW])
                        for m in range(2):
                            for h in range(4):
                                ps = psS.tile([128, CHW], f32, tag=f"s{h}",
                                              name=f"s{c}{i}{m}{h}")
                                nc.tensor.matmul(
                                    ps[:, 0:n],
                                    kproj[m][32 * h:32 * h + 32,
                                             i * 128:(i + 1) * 128],
                                    qproj[m][32 * h:32 * h + 32, lo:cs + CHW],
                                    start=True, stop=True,
                                    tile_position=(32 * h, 0))
                                if not causal:
                                    nc.vector.tensor_tensor(
                                        ps[:, 0:n], ps[:, 0:n], mT_sb[:, 0:n],
                                        OP.add)
                                e = epool.tile([128, CHW], bf16,
                                               tag=f"e{m}h{h}i{i}",
                                               name=f"e{m}h{h}i{i}c{c}")
                                etiles[(m, h, i)] = e
                                nc.scalar.activation(
                                    e[:, lo - cs:CHW], ps[:, 0:n], AF.Exp,
                                    scale=0.125)
                                if causal and 128 * i >= cs:
                                    off = 128 * i - cs
                                    nc.vector.tensor_tensor(
                                        e[:, off:off + 128],
                                        e[:, off:off + 128], triu_sb[:],
                                        OP.mult)
                    # A@V in outT form: o[0:64] = vv_h.T @ e, o[64] = colsum
                    for h in range(4):
                        om_ps = []
                        for om in range(2):
                            o = psAV.tile([128, CHW], f32, tag=f"o{om}",
                                          name=f"o{om}h{h}c{c}")
                            om_ps.append(o)
                            for x, i in enumerate(ilist):
                                lo = max(cs, 128 * i) if causal else cs
                                nc.tensor.matmul(
                                    o[0:65, lo - cs:CHW],
                                    vvo[:, 260 * i + 65 * h:
                                        260 * i + 65 * h + 65],
                                    etiles[(om, h, i)][:, lo - cs:CHW],
                                    start=(x == 0), stop=(x == len(ilist) - 1))
                        o1, o2 = om_ps
                        r1 = spool.tile([1, CHW], f32, tag="r1",
                                        name=f"r1h{h}c{c}")
                        nc.vector.reciprocal(r1[:], o1[64:65, 0:CHW])
                        r2 = spool.tile([1, CHW], f32, tag="r2",
                                        name=f"r2h{h}c{c}")
                        nc.vector.reciprocal(r2[:], o2[64:65, 0:CHW])
                        nc.vector.tensor_scalar(r2[:], r2[:],
                                                lam_sb[0:1, 0:1], None,
                                                OP.mult)
                        rb1 = spool.tile([64, CHW], f32, tag="rb1",
                                         name=f"rb1h{h}c{c}")
                        nc.gpsimd.partition_broadcast(rb1[:], r1[:])
                        rb2 = spool.tile([64, CHW], f32, tag="rb2",
                                         name=f"rb2h{h}c{c}")
                        nc.gpsimd.partition_broadcast(rb2[:], r2[:])
                        t1f = spool.tile([64, CHW], f32, tag="t1f",
                                         name=f"t1fh{h}c{c}")
                        t2f = spool.tile([64, CHW], f32, tag="t2f",
                                         name=f"t2fh{h}c{c}")
                        nc.vector.tensor_tensor(t1f[:], o1[0:64, 0:CHW],
                                                rb1[:], OP.mult)
                        nc.vector.tensor_tensor(t2f[:], o2[0:64, 0:CHW],
                                                rb2[:], OP.mult)
                        dst = aT_sb[h // 2][64 * (h % 2):64 * (h % 2) + 64,
                                            cs:cs + CHW]
                        nc.vector.tensor_tensor(dst, t1f[:], t2f[:],
                                                OP.subtract)
                    # ship this chunk's attnT and AllGather across all 8 cores
                    for kk in range(2):
                        nc.sync.dma_start(
                            bounce[c][128 * kk:128 * (kk + 1), :],
                            aT_sb[kk][:, cs:cs + CHW])
                    nc.gpsimd.collective_compute(
                        "AllGather", mybir.AluOpType.bypass,
                        replica_groups=RG8,
                        ins=[bounce[c].opt()], outs=[ag_out[c].opt()])
                    if c > 0:
                        wo_chunk(c - 1)
                wo_chunk(NCH - 1)

    nc.compile()
    return nc


def kernel(**inputs):
    global LAST_EXEC_NS
    import ml_dtypes

    q = np.asarray(inputs["q"], dtype=np.float32)
    k = np.asarray(inputs["k"], dtype=np.float32)
    v = np.asarray(inputs["v"], dtype=np.float32)
    mask = np.asarray(inputs["mask"])
    f32 = np.float32
    Wq1f = np.asarray(inputs["Wq1"], f32); Wq2f = np.asarray(inputs["Wq2"], f32)
    Wk1f = np.asarray(inputs["Wk1"], f32); Wk2f = np.asarray(inputs["Wk2"], f32)
    Wvf = np.asarray(inputs["Wv"], f32);   Wof = np.asarray(inputs["Wo"], f32)
    bq1f = np.asarray(inputs["bq1"], f32); bq2f = np.asarray(inputs["bq2"], f32)
    bk1f = np.asarray(inputs["bk1"], f32); bk2f = np.asarray(inputs["bk2"], f32)
    bvf = np.asarray(inputs["bv"], f32);   bof = np.asarray(inputs["bo"], f32)
    lam = float(np.exp(float(inputs["lq1"][0]) * float(inputs["lk1"][0]))
                - np.exp(float(inputs["lq2"][0]) * float(inputs["lk2"][0]))
                + LAMBDA_INIT)

    mk = (mask.reshape(B, S, S) != 0)
    causal = bool((mk == np.tril(np.ones((S, S), bool))[None]).all())

    key = "causal" if causal else "general"
    if key not in _cache:
        _cache[key] = _build(causal)
    nc = _cache[key]

    qT = [np.ascontiguousarray(q[b].T) for b in range(B)]
    kTl = [np.ascontiguousarray(k[b].T) for b in range(B)]
    vTl = [np.ascontiguousarray(v[b].T) for b in range(B)]
    Wob = Wof.astype(ml_dtypes.bfloat16)
    triu = np.triu(np.ones((128, 128))).astype(ml_dtypes.bfloat16)
    lamv = np.full((128, 1), lam, f32)
    maskTs = None
    if not causal:
        maskTs = [np.ascontiguousarray(
            np.where(mk[b], np.float32(0), np.float32(-1e9)).T)
            for b in range(B)]

    in_maps = []
    for c in range(NCORES):
        b, g = divmod(c, CPB)
        im = dict(
            qT=qT[b], kT=kTl[b], vT=vTl[b],
            Wq1=np.ascontiguousarray(Wq1f[:, 128 * g:128 * (g + 1)]),
            Wq2=np.ascontiguousarray(Wq2f[:, 128 * g:128 * (g + 1)]),
            Wk1=np.ascontiguousarray(Wk1f[:, 128 * g:128 * (g + 1)]),
            Wk2=np.ascontiguousarray(Wk2f[:, 128 * g:128 * (g + 1)]),
            Wv=np.ascontiguousarray(Wvf[:, 256 * g:256 * (g + 1)]),
            Wob=np.ascontiguousarray(Wob[:, 128 * c:128 * (c + 1)]),
            bq1=np.ascontiguousarray(bq1f[128 * g:128 * (g + 1)]).reshape(128, 1),
            bq2=np.ascontiguousarray(bq2f[128 * g:128 * (g + 1)]).reshape(128, 1),
            bk1=np.ascontiguousarray(bk1f[128 * g:128 * (g + 1)]).reshape(128, 1),
            bk2=np.ascontiguousarray(bk2f[128 * g:128 * (g + 1)]).reshape(128, 1),
            bv=np.ascontiguousarray(bvf[256 * g:256 * (g + 1)]).reshape(1, 256),
            triu=triu, lamv=lamv,
            ones1=np.ones((1, 128), f32),
        )
        if not causal:
            im["maskT"] = maskTs[b]
        in_maps.append(im)

    from concourse.bass_utils import run_bass_kernel_spmd
    if PROFILE:
        _try_install_ntff_hook()
        res = run_bass_kernel_spmd(nc, in_maps, list(range(NCORES)),
                                   trace=True)
        LAST_EXEC_NS = res.exec_time_ns
        globals()["LAST_RESULTS"] = res
    else:
        res = run_bass_kernel_spmd(nc, in_maps, list(range(NCORES)))

    out = np.empty((B, S, D), np.float32)
    for c in range(NCORES):
        o = res.results[c]["out"]
        for b in range(B):
            out[b, :, 128 * c:128 * (c + 1)] = o[b * S:(b + 1) * S, :]
    out += bof[None, None, :]
    return out
